# revision 27
# baseline (speedup 1.0000x reference)
"""Trainium2 Bass kernel for causal self-attention with LoRA on q/v.

Reference shapes: hidden_states [4, 2048, 1024], 16 heads x 64 dims,
LoRA rank 8 (scale 2.0) on q and v projections.

Sharding: 8 cores = 4 batches x 2 head-groups. Core c handles batch
c//2 and heads (c%2)*8 .. (c%2)*8+8, i.e. output channels
(c%2)*512 .. +512 of its batch. Each core's output is disjoint, so the
full output is assembled host-side with no device collectives.

Per-core kernel (all matmuls bf16, fp32 accumulation):
  - q^T/k^T projections:  [dh=128-chunk, t] = W_chunk^T.T @ x^T, LoRA and
    bias folded in (LoRA via an extra K=128 zero-padded matmul chunk,
    bias via the DVE epilogue's per-partition tensor_scalar add).
  - v projection in [t, dh] orientation (x^T chunks as stationary), LoRA
    + bias via one extra matmul chunk; epilogue scatters v into a
    [s-chunk, head, 65] buffer whose column 64 is constant 1.0.
  - attention per head, transposed: scores^T [s=128 block, t] in PSUM,
    exp on ScalarE with scale=1/8 and the additive attention mask as the
    per-partition bias; causal handled by skipping fully-masked blocks
    plus one [128,128] upper-triangular mask multiply per diagonal block.
  - PV: out[t-block, 0:64] += expS^T_chunk.T @ [v | 1]; column 64
    accumulates the softmax denominator. DVE reciprocal + scale, DMA out.
"""

import sys

if "/opt/trn_rl_repo" not in sys.path:
    sys.path.insert(0, "/opt/trn_rl_repo")

import numpy as np
import ml_dtypes

BF16 = ml_dtypes.bfloat16

B, T, H, NH, DH = 4, 2048, 1024, 16, 64
N_CORES = 8
HPC = 8          # heads per core
CH = HPC * DH    # 512 output channels per core
LORA_SCALE = 2.0

_cached = {}


def _build_nc():
    import concourse.bass as bass
    import concourse.mybir as mybir
    from concourse import bacc
    from concourse.tile import TileContext

    dt = mybir.dt
    AF = mybir.ActivationFunctionType

    nc = bacc.Bacc()

    xT_d = nc.dram_tensor("xT", [4, 128, 8, 512], dt.bfloat16, kind="ExternalInput")
    wqT_d = nc.dram_tensor("wqT", [128, 8, 512], dt.bfloat16, kind="ExternalInput")
    wkT_d = nc.dram_tensor("wkT", [128, 8, 512], dt.bfloat16, kind="ExternalInput")
    wvT_d = nc.dram_tensor("wvT", [128, 8, 512], dt.bfloat16, kind="ExternalInput")
    bqk_d = nc.dram_tensor("bqk", [128, 2, 4], dt.float32, kind="ExternalInput")
    loraA_d = nc.dram_tensor("loraA", [128, 8, 48], dt.bfloat16, kind="ExternalInput")
    qBsT_d = nc.dram_tensor("qBsT", [128, 512], dt.bfloat16, kind="ExternalInput")
    vBa_d = nc.dram_tensor("vBa", [128, 512], dt.bfloat16, kind="ExternalInput")
    amask_d = nc.dram_tensor("amask", [128, 16], dt.float32, kind="ExternalInput")
    tri_d = nc.dram_tensor("tri", [128, 128], dt.bfloat16, kind="ExternalInput")
    out_d = nc.dram_tensor("out", [T, CH], dt.float32, kind="ExternalOutput")

    with TileContext(nc) as tc:
        with (
            tc.tile_pool(name="const", bufs=1) as cpool,
            tc.tile_pool(name="big", bufs=1) as bpool,
            tc.tile_pool(name="small", bufs=4) as spool,
            tc.tile_pool(name="psproj", bufs=2, space="PSUM") as ps_proj,
            tc.tile_pool(name="pssc", bufs=1, space="PSUM") as ps_sc,
        ):
            # ---- persistent SBUF tensors -------------------------------
            amask_sb = cpool.tile([128, 16], dt.float32, tag="amask")
            nc.sync.dma_start(amask_sb[:], amask_d[:])
            tri_sb = cpool.tile([128, 128], dt.bfloat16, tag="tri")
            nc.sync.dma_start(tri_sb[:], tri_d[:])
            bqk_sb = cpool.tile([128, 2, 4], dt.float32, tag="bqk")
            nc.sync.dma_start(bqk_sb[:], bqk_d[:])
            loraA_sb = cpool.tile([128, 8, 48], dt.bfloat16, tag="loraA")
            nc.sync.dma_start(loraA_sb[:], loraA_d[:])
            qBsT_sb = cpool.tile([128, 512], dt.bfloat16, tag="qBsT")
            nc.sync.dma_start(qBsT_sb[:], qBsT_d[:])
            vBa_sb = cpool.tile([128, 512], dt.bfloat16, tag="vBa")
            nc.sync.dma_start(vBa_sb[:], vBa_d[:])

            x_sb = [[None] * 8 for _ in range(4)]
            def load_x(tb):
                for kc in range(8):
                    xt = bpool.tile(
                        [128, 512], dt.bfloat16, tag=f"x{tb}_{kc}", name=f"x{tb}_{kc}"
                    )
                    nc.sync.dma_start(xt[:], xT_d[tb, :, kc, :])
                    x_sb[tb][kc] = xt
            load_x(0)
            wq_sb = bpool.tile([128, 8, 512], dt.bfloat16, tag="wq")
            nc.sync.dma_start(wq_sb[:], wqT_d[:])
            wk_sb = bpool.tile([128, 8, 512], dt.bfloat16, tag="wk")
            nc.sync.dma_start(wk_sb[:], wkT_d[:])
            for tb in range(1, 4):
                load_x(tb)
            wv_sb = bpool.tile([128, 8, 512], dt.bfloat16, tag="wv")
            nc.sync.dma_start(wv_sb[:], wvT_d[:])

            # LoRA stage-1 outputs: rows 0-7 hold (A @ x^T) for q / v;
            # lv row 32 is the all-ones row that carries the v bias.
            lq_t, lv_t = [], []
            for tb in range(4):
                a = cpool.tile([128, 512], dt.bfloat16, tag=f"lq{tb}", name=f"lq{tb}")
                nc.gpsimd.memset(a[:], 0.0)
                lq_t.append(a)
                b = cpool.tile([128, 512], dt.bfloat16, tag=f"lv{tb}", name=f"lv{tb}")
                nc.gpsimd.memset(b[:], 0.0)
                nc.gpsimd.memset(b[32:33, :], 1.0)
                lv_t.append(b)

            qt = [
                [
                    bpool.tile([128, 512], dt.bfloat16, tag=f"q{j}_{tb}", name=f"qt{j}_{tb}")
                    for tb in range(4)
                ]
                for j in range(4)
            ]
            # k^T stored per head parity with the other 64 partitions zeroed,
            # so score matmuls run K=128 (other head's q rows hit zeros).
            kpt = []
            for j in range(4):
                pair = []
                for p in range(2):
                    row = []
                    for tb in range(4):
                        t = bpool.tile(
                            [128, 512], dt.bfloat16, tag=f"k{j}_{p}_{tb}",
                            name=f"kp{j}_{p}_{tb}",
                        )
                        if p == 0:
                            nc.vector.memset(t[64:128, :], 0.0)
                        else:
                            nc.vector.memset(t[0:64, :], 0.0)
                        row.append(t)
                    pair.append(row)
                kpt.append(pair)
            v_t = []
            for m in range(16):
                vt = bpool.tile([128, 8, 65], dt.bfloat16, tag=f"v{m}", name=f"v{m}")
                nc.gpsimd.memset(vt[:, :, 64:65], 1.0)
                v_t.append(vt)

            # ---- LoRA stage 1: [qA(0:8); vA(32:40)] @ x^T --------------
            for tb in range(4):
                pl = ps_proj.tile([48, 512], dt.float32, tag="proj", name="pl")
                for kc in range(8):
                    nc.tensor.matmul(
                        pl[:],
                        loraA_sb[:, kc, :],
                        x_sb[tb][kc][:],
                        start=(kc == 0),
                        stop=(kc == 7),
                    )
                nc.vector.tensor_copy(lq_t[tb][0:8, :], pl[0:8, :])
                nc.vector.tensor_copy(lv_t[tb][0:8, :], pl[32:40, :])

            # ---- q/k projections (transposed): [dh-chunk, t] -----------
            def proj_q_piece(j, tb):
                ms = slice(j * 128, (j + 1) * 128)
                pq = ps_proj.tile([128, 512], dt.float32, tag="proj", name="pq")
                for kc in range(8):
                    nc.tensor.matmul(
                        pq[:],
                        wq_sb[:, kc, ms],
                        x_sb[tb][kc][:],
                        start=(kc == 0),
                        stop=False,
                    )
                nc.tensor.matmul(
                    pq[:], qBsT_sb[:, ms], lq_t[tb][:], start=False, stop=True
                )
                nc.vector.tensor_scalar_add(qt[j][tb][:], pq[:], bqk_sb[:, 0, j : j + 1])

            def proj_k_piece(j, tb):
                ms = slice(j * 128, (j + 1) * 128)
                pk = ps_proj.tile([128, 512], dt.float32, tag="proj", name="pk")
                for kc in range(8):
                    nc.tensor.matmul(
                        pk[:],
                        wk_sb[:, kc, ms],
                        x_sb[tb][kc][:],
                        start=(kc == 0),
                        stop=(kc == 7),
                    )
                nc.vector.tensor_scalar_add(
                    kpt[j][0][tb][0:64, :], pk[0:64, :], bqk_sb[0:64, 1, j : j + 1]
                )
                nc.vector.tensor_scalar_add(
                    kpt[j][1][tb][64:128, :], pk[64:128, :], bqk_sb[64:128, 1, j : j + 1]
                )

            def qk_pieces(j):
                return [lambda tb=tb, j=j: proj_q_piece(j, tb) for tb in range(4)] + [
                    lambda tb=tb, j=j: proj_k_piece(j, tb) for tb in range(4)
                ]

            # ---- v projection: [t-chunk, dh] ---------------------------
            def proj_v(m):
                pv = ps_proj.tile([128, 512], dt.float32, tag="proj", name="pv")
                msl = slice((m % 4) * 128, (m % 4 + 1) * 128)
                for kc in range(8):
                    nc.tensor.matmul(
                        pv[:],
                        x_sb[m // 4][kc][:, msl],
                        wv_sb[:, kc, :],
                        start=(kc == 0),
                        stop=False,
                    )
                nc.tensor.matmul(
                    pv[:],
                    lv_t[m // 4][:, (m % 4) * 128 : (m % 4 + 1) * 128],
                    vBa_sb[:],
                    start=False,
                    stop=True,
                )
                nc.vector.tensor_copy(
                    v_t[m][:, :, 0:64], pv[:].rearrange("p (h d) -> p h d", h=8)
                )

            # ---- attention, single head, K=128 padded scores -----------
            def attn_scores(h, sb, exp_tiles):
                j, p = h // 2, h % 2
                w = 2048 - sb * 128
                ssl = slice(sb * 128, (sb + 1) * 128)
                et = spool.tile(
                    [128, w], dt.bfloat16, tag=f"e{sb}", name=f"et{sb}",
                    bufs=2,
                )
                exp_tiles.append(et)
                lhs = kpt[j][p][sb // 4][:, (sb % 4) * 128 : (sb % 4 + 1) * 128]
                diag_c = (sb * 128) // 512
                for ht in range(2):
                    c_lo = max(2 * ht, diag_c)
                    c_hi = 2 * ht + 2
                    if c_lo >= c_hi:
                        continue
                    sc = ps_sc.tile(
                        [128, 1024], dt.float32, tag=f"sc{(2 * sb + ht) % 3}", name="sc"
                    )
                    for c in range(c_lo, c_hi):
                        r = sb * 128 - c * 512 if c == diag_c else 0
                        ps0 = (c - 2 * ht) * 512 + r
                        nc.tensor.matmul(
                            sc[:, ps0 : (c - 2 * ht + 1) * 512],
                            lhs,
                            qt[j][c][:, r:512],
                            start=True,
                            stop=True,
                        )
                    off_in = max(0, sb * 128 - ht * 1024)
                    wv_ = 1024 - off_in
                    off_out = ht * 1024 + off_in - sb * 128
                    nc.scalar.activation(
                        et[:, off_out : off_out + wv_],
                        sc[:, off_in : 1024],
                        AF.Exp,
                        bias=amask_sb[:, sb : sb + 1],
                        scale=0.125,
                    )
                nc.gpsimd.tensor_mul(et[:, 0:128], et[:, 0:128], tri_sb[:])

            def attn_pv(h, m, exp_tiles):
                po_ps = ps_proj.tile([128, 65], dt.float32, tag="proj", name="po_ps")
                for s2 in range(m + 1):
                    off = (m - s2) * 128
                    nc.tensor.matmul(
                        po_ps[:],
                        exp_tiles[s2][:, off : off + 128],
                        v_t[s2][:, h, :],
                        start=(s2 == 0),
                        stop=(s2 == m),
                    )
                rz = spool.tile([128, 1], dt.float32, tag="rz", name="rz")
                nc.vector.reciprocal(rz[:], po_ps[:, 64:65])
                ot = spool.tile([128, 64], dt.float32, tag="ot", name="ot")
                nc.vector.tensor_scalar_mul(ot[:], po_ps[:, 0:64], rz[:])
                nc.sync.dma_start(
                    out_d[m * 128 : (m + 1) * 128, h * 64 : (h + 1) * 64], ot[:]
                )

            def attn_head(h, fillers=()):
                fq = list(fillers)
                exp_tiles = []
                for sb in range(3):
                    if fq:
                        fq.pop(0)()
                    attn_scores(h, sb, exp_tiles)
                for sb in range(3, 16):
                    if fq:
                        fq.pop(0)()
                    attn_pv(h, sb - 3, exp_tiles)
                    attn_scores(h, sb, exp_tiles)
                for f in fq:
                    f()
                for m in range(13, 16):
                    attn_pv(h, m, exp_tiles)

            def attn_head_split(h):
                exp_tiles = []
                for sb in range(16):
                    attn_scores(h, sb, exp_tiles)
                for m in range(16):
                    attn_pv(h, m, exp_tiles)

            for tb in range(4):
                proj_q_piece(0, tb)
            proj_k_piece(0, 0)
            # head 0: split schedule; remaining k pieces + v projection fill
            fill0 = (
                [lambda: proj_k_piece(0, 1), lambda: proj_k_piece(0, 2),
                 lambda: proj_k_piece(0, 3)]
                + [lambda m=m: proj_v(m) for m in range(16)]
            )
            et0 = []
            for sb in range(16):
                n = 2 if sb < 4 else 1
                for _ in range(n):
                    if fill0:
                        fill0.pop(0)()
                attn_scores(0, sb, et0)
            for f in fill0:
                f()
            for m in range(16):
                attn_pv(0, m, et0)
            attn_head(1, qk_pieces(1))
            attn_head(2)
            attn_head(3, qk_pieces(2))
            attn_head(4)
            attn_head(5, qk_pieces(3))
            attn_head(6)
            attn_head_split(7)

    nc.compile()
    return nc


def _prep_core_inputs(c, x, mask, Wq, bq, Wk, bk, Wv, bv, qA, qB, vA, vB):
    b, half = divmod(c, 2)
    hs = half * CH

    xT = np.ascontiguousarray(x[b].T.astype(BF16))  # [1024, 2048]
    xTd = np.ascontiguousarray(xT.reshape(8, 128, 4, 512).transpose(2, 1, 0, 3))

    def wT(W):
        Ws = W[hs : hs + CH]  # [512, 1024]
        return np.ascontiguousarray(
            Ws.T.astype(BF16).reshape(8, 128, 512).transpose(1, 0, 2)
        )

    bqk = np.ascontiguousarray(
        np.stack(
            [
                bq[hs : hs + CH].reshape(4, 128).T,
                bk[hs : hs + CH].reshape(4, 128).T,
            ],
            axis=1,
        ).astype(np.float32)
    )  # [128, 2, 4]

    A = np.zeros((48, H), np.float32)
    A[0:8] = qA
    A[32:40] = vA
    loraA = np.ascontiguousarray(
        A.T.astype(BF16).reshape(8, 128, 48).transpose(1, 0, 2)
    )

    qBsT = np.zeros((128, 512), BF16)
    qBsT[0:8] = (LORA_SCALE * qB[hs : hs + CH].T).astype(BF16)
    vBa = np.zeros((128, 512), BF16)
    vBa[0:8] = (LORA_SCALE * vB[hs : hs + CH].T).astype(BF16)
    vBa[32] = bv[hs : hs + CH].astype(BF16)

    amask = np.ascontiguousarray(
        mask[b, 0, 0].reshape(16, 128).T.astype(np.float32)
    )
    tri = np.triu(np.ones((128, 128), BF16))

    return {
        "xT": xTd,
        "wqT": wT(Wq),
        "wkT": wT(Wk),
        "wvT": wT(Wv),
        "bqk": bqk,
        "loraA": loraA,
        "qBsT": qBsT,
        "vBa": vBa,
        "amask": amask,
        "tri": tri,
    }


def _run(inputs, trace=False, trace_kwargs=None):
    from concourse.bass_utils import run_bass_kernel_spmd

    args = {k: np.asarray(v) for k, v in inputs.items()}
    in_maps = [
        _prep_core_inputs(
            c,
            args["hidden_states"],
            args["attention_mask"],
            args["Wq"], args["bq"], args["Wk"], args["bk"], args["Wv"], args["bv"],
            args["qA"], args["qB"], args["vA"], args["vB"],
        )
        for c in range(N_CORES)
    ]

    if "nc" not in _cached:
        _cached["nc"] = _build_nc()
    nc = _cached["nc"]

    res = run_bass_kernel_spmd(
        nc, in_maps, core_ids=list(range(N_CORES)), trace=trace,
        **(trace_kwargs or {}),
    )

    full = np.empty((B, T, H), np.float32)
    for c in range(N_CORES):
        b, half = divmod(c, 2)
        full[b, :, half * CH : (half + 1) * CH] = res.results[c]["out"]
    return full, res


def kernel(**inputs):
    full, _ = _run(inputs, trace=False)
    return full


# revision 28
# speedup vs baseline: 1.0042x; 1.0042x over previous
"""Trainium2 Bass kernel for causal self-attention with LoRA on q/v.

Reference shapes: hidden_states [4, 2048, 1024], 16 heads x 64 dims,
LoRA rank 8 (scale 2.0) on q and v projections.

Sharding: 8 cores = 4 batches x 2 head-groups. Core c handles batch
c//2 and heads (c%2)*8 .. (c%2)*8+8, i.e. output channels
(c%2)*512 .. +512 of its batch. Each core's output is disjoint, so the
full output is assembled host-side with no device collectives.

Per-core kernel (all matmuls bf16, fp32 accumulation):
  - q^T/k^T projections:  [dh=128-chunk, t] = W_chunk^T.T @ x^T, LoRA and
    bias folded in (LoRA via an extra K=128 zero-padded matmul chunk,
    bias via the DVE epilogue's per-partition tensor_scalar add).
  - v projection in [t, dh] orientation (x^T chunks as stationary), LoRA
    + bias via one extra matmul chunk; epilogue scatters v into a
    [s-chunk, head, 65] buffer whose column 64 is constant 1.0.
  - attention per head, transposed: scores^T [s=128 block, t] in PSUM,
    exp on ScalarE with scale=1/8 and the additive attention mask as the
    per-partition bias; causal handled by skipping fully-masked blocks
    plus one [128,128] upper-triangular mask multiply per diagonal block.
  - PV: out[t-block, 0:64] += expS^T_chunk.T @ [v | 1]; column 64
    accumulates the softmax denominator. DVE reciprocal + scale, DMA out.
"""

import sys

if "/opt/trn_rl_repo" not in sys.path:
    sys.path.insert(0, "/opt/trn_rl_repo")

import numpy as np
import ml_dtypes

BF16 = ml_dtypes.bfloat16

B, T, H, NH, DH = 4, 2048, 1024, 16, 64
N_CORES = 8
HPC = 8          # heads per core
CH = HPC * DH    # 512 output channels per core
LORA_SCALE = 2.0

_cached = {}


def _build_nc():
    import concourse.bass as bass
    import concourse.mybir as mybir
    from concourse import bacc
    from concourse.tile import TileContext

    dt = mybir.dt
    AF = mybir.ActivationFunctionType

    nc = bacc.Bacc()

    xT_d = nc.dram_tensor("xT", [4, 128, 8, 512], dt.bfloat16, kind="ExternalInput")
    wqT_d = nc.dram_tensor("wqT", [128, 8, 512], dt.bfloat16, kind="ExternalInput")
    wkT_d = nc.dram_tensor("wkT", [128, 8, 512], dt.bfloat16, kind="ExternalInput")
    wvT_d = nc.dram_tensor("wvT", [128, 8, 512], dt.bfloat16, kind="ExternalInput")
    bqk_d = nc.dram_tensor("bqk", [128, 2, 4], dt.float32, kind="ExternalInput")
    loraA_d = nc.dram_tensor("loraA", [128, 8, 48], dt.bfloat16, kind="ExternalInput")
    qBsT_d = nc.dram_tensor("qBsT", [128, 512], dt.bfloat16, kind="ExternalInput")
    vBa_d = nc.dram_tensor("vBa", [128, 512], dt.bfloat16, kind="ExternalInput")
    amask_d = nc.dram_tensor("amask", [128, 16], dt.float32, kind="ExternalInput")
    tri_d = nc.dram_tensor("tri", [128, 128], dt.bfloat16, kind="ExternalInput")
    out_d = nc.dram_tensor("out", [T, CH], dt.float32, kind="ExternalOutput")

    with TileContext(nc) as tc:
        with (
            tc.tile_pool(name="const", bufs=1) as cpool,
            tc.tile_pool(name="big", bufs=1) as bpool,
            tc.tile_pool(name="small", bufs=4) as spool,
            tc.tile_pool(name="psproj", bufs=2, space="PSUM") as ps_proj,
            tc.tile_pool(name="pssc", bufs=1, space="PSUM") as ps_sc,
        ):
            # ---- persistent SBUF tensors -------------------------------
            amask_sb = cpool.tile([128, 16], dt.float32, tag="amask")
            nc.sync.dma_start(amask_sb[:], amask_d[:])
            tri_sb = cpool.tile([128, 128], dt.bfloat16, tag="tri")
            nc.sync.dma_start(tri_sb[:], tri_d[:])
            bqk_sb = cpool.tile([128, 2, 4], dt.float32, tag="bqk")
            nc.sync.dma_start(bqk_sb[:], bqk_d[:])
            loraA_sb = cpool.tile([128, 8, 48], dt.bfloat16, tag="loraA")
            nc.sync.dma_start(loraA_sb[:], loraA_d[:])
            qBsT_sb = cpool.tile([128, 512], dt.bfloat16, tag="qBsT")
            nc.sync.dma_start(qBsT_sb[:], qBsT_d[:])
            vBa_sb = cpool.tile([128, 512], dt.bfloat16, tag="vBa")
            nc.sync.dma_start(vBa_sb[:], vBa_d[:])

            x_sb = [[None] * 8 for _ in range(4)]
            def load_x(tb):
                for kc in range(8):
                    xt = bpool.tile(
                        [128, 512], dt.bfloat16, tag=f"x{tb}_{kc}", name=f"x{tb}_{kc}"
                    )
                    nc.sync.dma_start(xt[:], xT_d[tb, :, kc, :])
                    x_sb[tb][kc] = xt
            load_x(0)
            wq_sb = bpool.tile([128, 8, 512], dt.bfloat16, tag="wq")
            nc.sync.dma_start(wq_sb[:], wqT_d[:])
            wk_sb = bpool.tile([128, 8, 512], dt.bfloat16, tag="wk")
            nc.sync.dma_start(wk_sb[:], wkT_d[:])
            for tb in range(1, 4):
                load_x(tb)
            wv_sb = bpool.tile([128, 8, 512], dt.bfloat16, tag="wv")
            nc.sync.dma_start(wv_sb[:], wvT_d[:])

            # LoRA stage-1 outputs: rows 0-7 hold (A @ x^T) for q / v;
            # lv row 32 is the all-ones row that carries the v bias.
            lq_t, lv_t = [], []
            for tb in range(4):
                a = cpool.tile([128, 512], dt.bfloat16, tag=f"lq{tb}", name=f"lq{tb}")
                nc.gpsimd.memset(a[:], 0.0)
                lq_t.append(a)
                b = cpool.tile([128, 512], dt.bfloat16, tag=f"lv{tb}", name=f"lv{tb}")
                nc.gpsimd.memset(b[:], 0.0)
                nc.gpsimd.memset(b[32:33, :], 1.0)
                lv_t.append(b)

            qt = [
                [
                    bpool.tile([128, 512], dt.bfloat16, tag=f"q{j}_{tb}", name=f"qt{j}_{tb}")
                    for tb in range(4)
                ]
                for j in range(4)
            ]
            # k^T stored per head parity with the other 64 partitions zeroed,
            # so score matmuls run K=128 (other head's q rows hit zeros).
            kpt = []
            for j in range(4):
                pair = []
                for p in range(2):
                    row = []
                    for tb in range(4):
                        t = bpool.tile(
                            [128, 512], dt.bfloat16, tag=f"k{j}_{p}_{tb}",
                            name=f"kp{j}_{p}_{tb}",
                        )
                        if p == 0:
                            nc.vector.memset(t[64:128, :], 0.0)
                        else:
                            nc.vector.memset(t[0:64, :], 0.0)
                        row.append(t)
                    pair.append(row)
                kpt.append(pair)
            v_t = []
            for m in range(16):
                vt = bpool.tile([128, 8, 65], dt.bfloat16, tag=f"v{m}", name=f"v{m}")
                nc.gpsimd.memset(vt[:, :, 64:65], 1.0)
                v_t.append(vt)

            # ---- LoRA stage 1: [qA(0:8); vA(32:40)] @ x^T --------------
            for tb in range(4):
                pl = ps_proj.tile([48, 512], dt.float32, tag="proj", name="pl")
                for kc in range(8):
                    nc.tensor.matmul(
                        pl[:],
                        loraA_sb[:, kc, :],
                        x_sb[tb][kc][:],
                        start=(kc == 0),
                        stop=(kc == 7),
                    )
                nc.vector.tensor_copy(lq_t[tb][0:8, :], pl[0:8, :])
                nc.vector.tensor_copy(lv_t[tb][0:8, :], pl[32:40, :])

            # ---- q/k projections (transposed): [dh-chunk, t] -----------
            def proj_q_piece(j, tb):
                ms = slice(j * 128, (j + 1) * 128)
                pq = ps_proj.tile([128, 512], dt.float32, tag="proj", name="pq")
                for kc in range(8):
                    nc.tensor.matmul(
                        pq[:],
                        wq_sb[:, kc, ms],
                        x_sb[tb][kc][:],
                        start=(kc == 0),
                        stop=False,
                    )
                nc.tensor.matmul(
                    pq[:], qBsT_sb[:, ms], lq_t[tb][:], start=False, stop=True
                )
                nc.vector.tensor_scalar_add(qt[j][tb][:], pq[:], bqk_sb[:, 0, j : j + 1])

            def proj_k_piece(j, tb):
                ms = slice(j * 128, (j + 1) * 128)
                pk = ps_proj.tile([128, 512], dt.float32, tag="proj", name="pk")
                for kc in range(8):
                    nc.tensor.matmul(
                        pk[:],
                        wk_sb[:, kc, ms],
                        x_sb[tb][kc][:],
                        start=(kc == 0),
                        stop=(kc == 7),
                    )
                nc.vector.tensor_scalar_add(
                    kpt[j][0][tb][0:64, :], pk[0:64, :], bqk_sb[0:64, 1, j : j + 1]
                )
                nc.vector.tensor_scalar_add(
                    kpt[j][1][tb][64:128, :], pk[64:128, :], bqk_sb[64:128, 1, j : j + 1]
                )

            def qk_pieces(j):
                return [lambda tb=tb, j=j: proj_q_piece(j, tb) for tb in range(4)] + [
                    lambda tb=tb, j=j: proj_k_piece(j, tb) for tb in range(4)
                ]

            # ---- v projection: [t-chunk, dh] ---------------------------
            def proj_v(m):
                pv = ps_proj.tile([128, 512], dt.float32, tag="proj", name="pv")
                msl = slice((m % 4) * 128, (m % 4 + 1) * 128)
                for kc in range(8):
                    nc.tensor.matmul(
                        pv[:],
                        x_sb[m // 4][kc][:, msl],
                        wv_sb[:, kc, :],
                        start=(kc == 0),
                        stop=False,
                    )
                nc.tensor.matmul(
                    pv[:],
                    lv_t[m // 4][:, (m % 4) * 128 : (m % 4 + 1) * 128],
                    vBa_sb[:],
                    start=False,
                    stop=True,
                )
                nc.vector.tensor_copy(
                    v_t[m][:, :, 0:64], pv[:].rearrange("p (h d) -> p h d", h=8)
                )

            # ---- attention, single head, K=128 padded scores -----------
            def attn_scores(h, sb, exp_tiles):
                j, p = h // 2, h % 2
                w = 2048 - sb * 128
                ssl = slice(sb * 128, (sb + 1) * 128)
                et = spool.tile(
                    [128, w], dt.bfloat16, tag=f"e{sb}", name=f"et{sb}",
                    bufs=2,
                )
                exp_tiles.append(et)
                lhs = kpt[j][p][sb // 4][:, (sb % 4) * 128 : (sb % 4 + 1) * 128]
                diag_c = (sb * 128) // 512
                for ht in range(2):
                    c_lo = max(2 * ht, diag_c)
                    c_hi = 2 * ht + 2
                    if c_lo >= c_hi:
                        continue
                    sc = ps_sc.tile(
                        [128, 1024], dt.float32, tag=f"sc{(2 * sb + ht) % 3}", name="sc"
                    )
                    for c in range(c_lo, c_hi):
                        r = sb * 128 - c * 512 if c == diag_c else 0
                        ps0 = (c - 2 * ht) * 512 + r
                        nc.tensor.matmul(
                            sc[:, ps0 : (c - 2 * ht + 1) * 512],
                            lhs,
                            qt[j][c][:, r:512],
                            start=True,
                            stop=True,
                        )
                    off_in = max(0, sb * 128 - ht * 1024)
                    wv_ = 1024 - off_in
                    off_out = ht * 1024 + off_in - sb * 128
                    nc.scalar.activation(
                        et[:, off_out : off_out + wv_],
                        sc[:, off_in : 1024],
                        AF.Exp,
                        bias=amask_sb[:, sb : sb + 1],
                        scale=0.125,
                    )
                nc.vector.tensor_mul(et[:, 0:128], et[:, 0:128], tri_sb[:])

            def attn_pv(h, m, exp_tiles):
                po_ps = ps_proj.tile([128, 65], dt.float32, tag="proj", name="po_ps")
                for s2 in range(m + 1):
                    off = (m - s2) * 128
                    nc.tensor.matmul(
                        po_ps[:],
                        exp_tiles[s2][:, off : off + 128],
                        v_t[s2][:, h, :],
                        start=(s2 == 0),
                        stop=(s2 == m),
                    )
                rz = spool.tile([128, 1], dt.float32, tag="rz", name="rz")
                nc.vector.reciprocal(rz[:], po_ps[:, 64:65])
                ot = spool.tile([128, 64], dt.float32, tag="ot", name="ot")
                nc.vector.tensor_scalar_mul(ot[:], po_ps[:, 0:64], rz[:])
                nc.sync.dma_start(
                    out_d[m * 128 : (m + 1) * 128, h * 64 : (h + 1) * 64], ot[:]
                )

            def attn_head(h, fillers=()):
                fq = list(fillers)
                exp_tiles = []
                for sb in range(3):
                    if fq:
                        fq.pop(0)()
                    attn_scores(h, sb, exp_tiles)
                for sb in range(3, 16):
                    if fq:
                        fq.pop(0)()
                    attn_pv(h, sb - 3, exp_tiles)
                    attn_scores(h, sb, exp_tiles)
                for f in fq:
                    f()
                for m in range(13, 16):
                    attn_pv(h, m, exp_tiles)

            def attn_head_split(h):
                exp_tiles = []
                for sb in range(16):
                    attn_scores(h, sb, exp_tiles)
                for m in range(16):
                    attn_pv(h, m, exp_tiles)

            for tb in range(4):
                proj_q_piece(0, tb)
            proj_k_piece(0, 0)
            # head 0: split schedule; remaining k pieces + v projection fill
            fill0 = (
                [lambda: proj_k_piece(0, 1), lambda: proj_k_piece(0, 2),
                 lambda: proj_k_piece(0, 3)]
                + [lambda m=m: proj_v(m) for m in range(16)]
            )
            et0 = []
            for sb in range(16):
                n = 2 if sb < 4 else 1
                for _ in range(n):
                    if fill0:
                        fill0.pop(0)()
                attn_scores(0, sb, et0)
            for f in fill0:
                f()
            for m in range(16):
                attn_pv(0, m, et0)
            attn_head(1, qk_pieces(1))
            attn_head(2)
            attn_head(3, qk_pieces(2))
            attn_head(4)
            attn_head(5, qk_pieces(3))
            attn_head(6)
            attn_head_split(7)

    nc.compile()
    return nc


def _prep_core_inputs(c, x, mask, Wq, bq, Wk, bk, Wv, bv, qA, qB, vA, vB):
    b, half = divmod(c, 2)
    hs = half * CH

    xT = np.ascontiguousarray(x[b].T.astype(BF16))  # [1024, 2048]
    xTd = np.ascontiguousarray(xT.reshape(8, 128, 4, 512).transpose(2, 1, 0, 3))

    def wT(W):
        Ws = W[hs : hs + CH]  # [512, 1024]
        return np.ascontiguousarray(
            Ws.T.astype(BF16).reshape(8, 128, 512).transpose(1, 0, 2)
        )

    bqk = np.ascontiguousarray(
        np.stack(
            [
                bq[hs : hs + CH].reshape(4, 128).T,
                bk[hs : hs + CH].reshape(4, 128).T,
            ],
            axis=1,
        ).astype(np.float32)
    )  # [128, 2, 4]

    A = np.zeros((48, H), np.float32)
    A[0:8] = qA
    A[32:40] = vA
    loraA = np.ascontiguousarray(
        A.T.astype(BF16).reshape(8, 128, 48).transpose(1, 0, 2)
    )

    qBsT = np.zeros((128, 512), BF16)
    qBsT[0:8] = (LORA_SCALE * qB[hs : hs + CH].T).astype(BF16)
    vBa = np.zeros((128, 512), BF16)
    vBa[0:8] = (LORA_SCALE * vB[hs : hs + CH].T).astype(BF16)
    vBa[32] = bv[hs : hs + CH].astype(BF16)

    amask = np.ascontiguousarray(
        mask[b, 0, 0].reshape(16, 128).T.astype(np.float32)
    )
    tri = np.triu(np.ones((128, 128), BF16))

    return {
        "xT": xTd,
        "wqT": wT(Wq),
        "wkT": wT(Wk),
        "wvT": wT(Wv),
        "bqk": bqk,
        "loraA": loraA,
        "qBsT": qBsT,
        "vBa": vBa,
        "amask": amask,
        "tri": tri,
    }


def _run(inputs, trace=False, trace_kwargs=None):
    from concourse.bass_utils import run_bass_kernel_spmd

    args = {k: np.asarray(v) for k, v in inputs.items()}
    in_maps = [
        _prep_core_inputs(
            c,
            args["hidden_states"],
            args["attention_mask"],
            args["Wq"], args["bq"], args["Wk"], args["bk"], args["Wv"], args["bv"],
            args["qA"], args["qB"], args["vA"], args["vB"],
        )
        for c in range(N_CORES)
    ]

    if "nc" not in _cached:
        _cached["nc"] = _build_nc()
    nc = _cached["nc"]

    res = run_bass_kernel_spmd(
        nc, in_maps, core_ids=list(range(N_CORES)), trace=trace,
        **(trace_kwargs or {}),
    )

    full = np.empty((B, T, H), np.float32)
    for c in range(N_CORES):
        b, half = divmod(c, 2)
        full[b, :, half * CH : (half + 1) * CH] = res.results[c]["out"]
    return full, res


def kernel(**inputs):
    full, _ = _run(inputs, trace=False)
    return full


# revision 29
# speedup vs baseline: 1.0588x; 1.0543x over previous
"""Trainium2 Bass kernel for causal self-attention with LoRA on q/v.

Reference shapes: hidden_states [4, 2048, 1024], 16 heads x 64 dims,
LoRA rank 8 (scale 2.0) on q and v projections.

Sharding: 8 cores = 4 batches x 2 head-groups. Core c handles batch
c//2 and heads (c%2)*8 .. (c%2)*8+8, i.e. output channels
(c%2)*512 .. +512 of its batch. Each core's output is disjoint, so the
full output is assembled host-side with no device collectives.

Per-core kernel (all matmuls bf16, fp32 accumulation):
  - q^T/k^T projections:  [dh=128-chunk, t] = W_chunk^T.T @ x^T, LoRA and
    bias folded in (LoRA via an extra K=128 zero-padded matmul chunk,
    bias via the DVE epilogue's per-partition tensor_scalar add).
  - v projection in [t, dh] orientation (x^T chunks as stationary), LoRA
    + bias via one extra matmul chunk; epilogue scatters v into a
    [s-chunk, head, 65] buffer whose column 64 is constant 1.0.
  - attention per head, transposed: scores^T [s=128 block, t] in PSUM,
    exp on ScalarE with scale=1/8 and the additive attention mask as the
    per-partition bias; causal handled by skipping fully-masked blocks
    plus one [128,128] upper-triangular mask multiply per diagonal block.
  - PV: out[t-block, 0:64] += expS^T_chunk.T @ [v | 1]; column 64
    accumulates the softmax denominator. DVE reciprocal + scale, DMA out.
"""

import sys

if "/opt/trn_rl_repo" not in sys.path:
    sys.path.insert(0, "/opt/trn_rl_repo")

import numpy as np
import ml_dtypes

BF16 = ml_dtypes.bfloat16

B, T, H, NH, DH = 4, 2048, 1024, 16, 64
N_CORES = 8
HPC = 8          # heads per core
CH = HPC * DH    # 512 output channels per core
LORA_SCALE = 2.0

_cached = {}


def _build_nc():
    import concourse.bass as bass
    import concourse.mybir as mybir
    from concourse import bacc
    from concourse.tile import TileContext

    dt = mybir.dt
    AF = mybir.ActivationFunctionType

    nc = bacc.Bacc()

    xT_d = nc.dram_tensor("xT", [4, 128, 8, 512], dt.bfloat16, kind="ExternalInput")
    wqT_d = nc.dram_tensor("wqT", [128, 8, 512], dt.bfloat16, kind="ExternalInput")
    wkT_d = nc.dram_tensor("wkT", [128, 8, 512], dt.bfloat16, kind="ExternalInput")
    wvT_d = nc.dram_tensor("wvT", [128, 8, 512], dt.bfloat16, kind="ExternalInput")
    bqk_d = nc.dram_tensor("bqk", [128, 2, 4], dt.float32, kind="ExternalInput")
    loraA_d = nc.dram_tensor("loraA", [128, 8, 48], dt.bfloat16, kind="ExternalInput")
    qBsT_d = nc.dram_tensor("qBsT", [128, 512], dt.bfloat16, kind="ExternalInput")
    vBa_d = nc.dram_tensor("vBa", [128, 512], dt.bfloat16, kind="ExternalInput")
    amask_d = nc.dram_tensor("amask", [128, 16], dt.float32, kind="ExternalInput")
    tri_d = nc.dram_tensor("tri", [128, 128], dt.bfloat16, kind="ExternalInput")
    out_d = nc.dram_tensor("out", [T, CH], dt.float32, kind="ExternalOutput")

    with TileContext(nc) as tc:
        with (
            tc.tile_pool(name="const", bufs=1) as cpool,
            tc.tile_pool(name="big", bufs=1) as bpool,
            tc.tile_pool(name="small", bufs=4) as spool,
            tc.tile_pool(name="psproj", bufs=2, space="PSUM") as ps_proj,
            tc.tile_pool(name="pssc", bufs=1, space="PSUM") as ps_sc,
        ):
            # ---- persistent SBUF tensors -------------------------------
            amask_sb = cpool.tile([128, 16], dt.float32, tag="amask")
            nc.sync.dma_start(amask_sb[:], amask_d[:])
            tri_sb = cpool.tile([128, 128], dt.bfloat16, tag="tri")
            nc.sync.dma_start(tri_sb[:], tri_d[:])
            bqk_sb = cpool.tile([128, 2, 4], dt.float32, tag="bqk")
            nc.sync.dma_start(bqk_sb[:], bqk_d[:])
            loraA_sb = cpool.tile([128, 8, 48], dt.bfloat16, tag="loraA")
            nc.sync.dma_start(loraA_sb[:], loraA_d[:])
            qBsT_sb = cpool.tile([128, 512], dt.bfloat16, tag="qBsT")
            nc.sync.dma_start(qBsT_sb[:], qBsT_d[:])
            vBa_sb = cpool.tile([128, 512], dt.bfloat16, tag="vBa")
            nc.sync.dma_start(vBa_sb[:], vBa_d[:])

            x_sb = [[None] * 8 for _ in range(4)]
            def load_x(tb):
                for kc in range(8):
                    xt = bpool.tile(
                        [128, 512], dt.bfloat16, tag=f"x{tb}_{kc}", name=f"x{tb}_{kc}"
                    )
                    nc.sync.dma_start(xt[:], xT_d[tb, :, kc, :])
                    x_sb[tb][kc] = xt
            load_x(0)
            wq_sb = bpool.tile([128, 8, 512], dt.bfloat16, tag="wq")
            nc.sync.dma_start(wq_sb[:], wqT_d[:])
            wk_sb = bpool.tile([128, 8, 512], dt.bfloat16, tag="wk")
            nc.sync.dma_start(wk_sb[:], wkT_d[:])
            for tb in range(1, 4):
                load_x(tb)
            wv_sb = bpool.tile([128, 8, 512], dt.bfloat16, tag="wv")
            nc.sync.dma_start(wv_sb[:], wvT_d[:])

            # LoRA stage-1 outputs: rows 0-7 hold (A @ x^T) for q / v;
            # lv row 32 is the all-ones row that carries the v bias.
            lq_t, lv_t = [], []
            for tb in range(4):
                a = cpool.tile([128, 512], dt.bfloat16, tag=f"lq{tb}", name=f"lq{tb}")
                nc.gpsimd.memset(a[:], 0.0)
                lq_t.append(a)
                b = cpool.tile([128, 512], dt.bfloat16, tag=f"lv{tb}", name=f"lv{tb}")
                nc.gpsimd.memset(b[:], 0.0)
                nc.gpsimd.memset(b[32:33, :], 1.0)
                lv_t.append(b)

            qt = [
                [
                    bpool.tile([128, 512], dt.bfloat16, tag=f"q{j}_{tb}", name=f"qt{j}_{tb}")
                    for tb in range(4)
                ]
                for j in range(4)
            ]
            # k^T stored per head parity with the other 64 partitions zeroed,
            # so score matmuls run K=128 (other head's q rows hit zeros).
            kpt = []
            for j in range(4):
                pair = []
                for p in range(2):
                    row = []
                    for tb in range(4):
                        t = bpool.tile(
                            [128, 512], dt.bfloat16, tag=f"k{j}_{p}_{tb}",
                            name=f"kp{j}_{p}_{tb}",
                        )
                        if p == 0:
                            nc.vector.memset(t[64:128, :], 0.0)
                        else:
                            nc.vector.memset(t[0:64, :], 0.0)
                        row.append(t)
                    pair.append(row)
                kpt.append(pair)
            v_t = []
            for m in range(16):
                vt = bpool.tile([128, 8, 65], dt.bfloat16, tag=f"v{m}", name=f"v{m}")
                nc.gpsimd.memset(vt[:, :, 64:65], 1.0)
                v_t.append(vt)

            # ---- LoRA stage 1: [qA(0:8); vA(32:40)] @ x^T --------------
            def lora1_piece(tb):
                pl = ps_proj.tile([48, 512], dt.float32, tag="proj", name="pl")
                for kc in range(8):
                    nc.tensor.matmul(
                        pl[:],
                        loraA_sb[:, kc, :],
                        x_sb[tb][kc][:],
                        start=(kc == 0),
                        stop=(kc == 7),
                    )
                nc.vector.tensor_copy(lq_t[tb][0:8, :], pl[0:8, :])
                nc.vector.tensor_copy(lv_t[tb][0:8, :], pl[32:40, :])

            # ---- q/k projections (transposed): [dh-chunk, t] -----------
            def proj_q_piece(j, tb):
                ms = slice(j * 128, (j + 1) * 128)
                pq = ps_proj.tile([128, 512], dt.float32, tag="proj", name="pq")
                for kc in range(8):
                    nc.tensor.matmul(
                        pq[:],
                        wq_sb[:, kc, ms],
                        x_sb[tb][kc][:],
                        start=(kc == 0),
                        stop=False,
                    )
                nc.tensor.matmul(
                    pq[:], qBsT_sb[:, ms], lq_t[tb][:], start=False, stop=True
                )
                nc.vector.tensor_scalar_add(qt[j][tb][:], pq[:], bqk_sb[:, 0, j : j + 1])

            def proj_k_piece(j, tb):
                ms = slice(j * 128, (j + 1) * 128)
                pk = ps_proj.tile([128, 512], dt.float32, tag="proj", name="pk")
                for kc in range(8):
                    nc.tensor.matmul(
                        pk[:],
                        wk_sb[:, kc, ms],
                        x_sb[tb][kc][:],
                        start=(kc == 0),
                        stop=(kc == 7),
                    )
                nc.vector.tensor_scalar_add(
                    kpt[j][0][tb][0:64, :], pk[0:64, :], bqk_sb[0:64, 1, j : j + 1]
                )
                nc.vector.tensor_scalar_add(
                    kpt[j][1][tb][64:128, :], pk[64:128, :], bqk_sb[64:128, 1, j : j + 1]
                )

            def qk_pieces(j):
                return [lambda tb=tb, j=j: proj_q_piece(j, tb) for tb in range(4)] + [
                    lambda tb=tb, j=j: proj_k_piece(j, tb) for tb in range(4)
                ]

            # ---- v projection: [t-chunk, dh] ---------------------------
            def proj_v(m):
                pv = ps_proj.tile([128, 512], dt.float32, tag="proj", name="pv")
                msl = slice((m % 4) * 128, (m % 4 + 1) * 128)
                for kc in range(8):
                    nc.tensor.matmul(
                        pv[:],
                        x_sb[m // 4][kc][:, msl],
                        wv_sb[:, kc, :],
                        start=(kc == 0),
                        stop=False,
                    )
                nc.tensor.matmul(
                    pv[:],
                    lv_t[m // 4][:, (m % 4) * 128 : (m % 4 + 1) * 128],
                    vBa_sb[:],
                    start=False,
                    stop=True,
                )
                nc.vector.tensor_copy(
                    v_t[m][:, :, 0:64], pv[:].rearrange("p (h d) -> p h d", h=8)
                )

            # ---- attention, single head, K=128 padded scores -----------
            def attn_scores(h, sb, exp_tiles):
                j, p = h // 2, h % 2
                w = 2048 - sb * 128
                ssl = slice(sb * 128, (sb + 1) * 128)
                et = spool.tile(
                    [128, w], dt.bfloat16, tag=f"e{sb}", name=f"et{sb}",
                    bufs=2,
                )
                exp_tiles.append(et)
                lhs = kpt[j][p][sb // 4][:, (sb % 4) * 128 : (sb % 4 + 1) * 128]
                diag_c = (sb * 128) // 512
                for ht in range(2):
                    c_lo = max(2 * ht, diag_c)
                    c_hi = 2 * ht + 2
                    if c_lo >= c_hi:
                        continue
                    sc = ps_sc.tile(
                        [128, 1024], dt.float32, tag=f"sc{(2 * sb + ht) % 3}", name="sc"
                    )
                    for c in range(c_lo, c_hi):
                        r = sb * 128 - c * 512 if c == diag_c else 0
                        ps0 = (c - 2 * ht) * 512 + r
                        nc.tensor.matmul(
                            sc[:, ps0 : (c - 2 * ht + 1) * 512],
                            lhs,
                            qt[j][c][:, r:512],
                            start=True,
                            stop=True,
                        )
                    off_in = max(0, sb * 128 - ht * 1024)
                    wv_ = 1024 - off_in
                    off_out = ht * 1024 + off_in - sb * 128
                    nc.scalar.activation(
                        et[:, off_out : off_out + wv_],
                        sc[:, off_in : 1024],
                        AF.Exp,
                        bias=amask_sb[:, sb : sb + 1],
                        scale=0.125,
                    )
                nc.vector.tensor_mul(et[:, 0:128], et[:, 0:128], tri_sb[:])

            def attn_pv(h, m, exp_tiles):
                po_ps = ps_proj.tile([128, 65], dt.float32, tag="proj", name="po_ps")
                for s2 in range(m + 1):
                    off = (m - s2) * 128
                    nc.tensor.matmul(
                        po_ps[:],
                        exp_tiles[s2][:, off : off + 128],
                        v_t[s2][:, h, :],
                        start=(s2 == 0),
                        stop=(s2 == m),
                    )
                rz = spool.tile([128, 1], dt.float32, tag="rz", name="rz")
                nc.vector.reciprocal(rz[:], po_ps[:, 64:65])
                ot = spool.tile([128, 64], dt.float32, tag="ot", name="ot")
                nc.vector.tensor_scalar_mul(ot[:], po_ps[:, 0:64], rz[:])
                nc.sync.dma_start(
                    out_d[m * 128 : (m + 1) * 128, h * 64 : (h + 1) * 64], ot[:]
                )

            def attn_head(h, fillers=()):
                fq = list(fillers)
                exp_tiles = []
                for sb in range(3):
                    if fq and sb % 2 == 0:
                        fq.pop(0)()
                    attn_scores(h, sb, exp_tiles)
                for sb in range(3, 16):
                    if fq and sb % 2 == 0:
                        fq.pop(0)()
                    attn_pv(h, sb - 3, exp_tiles)
                    attn_scores(h, sb, exp_tiles)
                for f in fq:
                    f()
                for m in range(13, 16):
                    attn_pv(h, m, exp_tiles)

            def attn_head_split(h):
                exp_tiles = []
                for sb in range(16):
                    attn_scores(h, sb, exp_tiles)
                for m in range(16):
                    attn_pv(h, m, exp_tiles)

            for tb in range(4):
                lora1_piece(tb)
                proj_q_piece(0, tb)
            proj_k_piece(0, 0)
            # head 0: split schedule; remaining k pieces + v projection fill
            fill0 = (
                [lambda: proj_k_piece(0, 1), lambda: proj_k_piece(0, 2),
                 lambda: proj_k_piece(0, 3)]
                + [lambda m=m: proj_v(m) for m in range(16)]
            )
            et0 = []
            for sb in range(16):
                n = 2 if sb < 4 else 1
                for _ in range(n):
                    if fill0:
                        fill0.pop(0)()
                attn_scores(0, sb, et0)
            for f in fill0:
                f()
            for m in range(16):
                attn_pv(0, m, et0)
            def qf(j):
                return [lambda tb=tb, j=j: proj_q_piece(j, tb) for tb in range(4)] + [
                    lambda j=j: proj_k_piece(j, 0)
                ]

            def kf(j):
                return [lambda tb=tb, j=j: proj_k_piece(j, tb) for tb in (1, 2, 3)]

            attn_head(1, qf(1))
            attn_head(2, kf(1))
            attn_head(3, qf(2))
            attn_head(4, kf(2))
            attn_head(5, qf(3))
            attn_head(6, kf(3))
            attn_head_split(7)

    nc.compile()
    return nc


def _prep_core_inputs(c, x, mask, Wq, bq, Wk, bk, Wv, bv, qA, qB, vA, vB):
    b, half = divmod(c, 2)
    hs = half * CH

    xT = np.ascontiguousarray(x[b].T.astype(BF16))  # [1024, 2048]
    xTd = np.ascontiguousarray(xT.reshape(8, 128, 4, 512).transpose(2, 1, 0, 3))

    def wT(W):
        Ws = W[hs : hs + CH]  # [512, 1024]
        return np.ascontiguousarray(
            Ws.T.astype(BF16).reshape(8, 128, 512).transpose(1, 0, 2)
        )

    bqk = np.ascontiguousarray(
        np.stack(
            [
                bq[hs : hs + CH].reshape(4, 128).T,
                bk[hs : hs + CH].reshape(4, 128).T,
            ],
            axis=1,
        ).astype(np.float32)
    )  # [128, 2, 4]

    A = np.zeros((48, H), np.float32)
    A[0:8] = qA
    A[32:40] = vA
    loraA = np.ascontiguousarray(
        A.T.astype(BF16).reshape(8, 128, 48).transpose(1, 0, 2)
    )

    qBsT = np.zeros((128, 512), BF16)
    qBsT[0:8] = (LORA_SCALE * qB[hs : hs + CH].T).astype(BF16)
    vBa = np.zeros((128, 512), BF16)
    vBa[0:8] = (LORA_SCALE * vB[hs : hs + CH].T).astype(BF16)
    vBa[32] = bv[hs : hs + CH].astype(BF16)

    amask = np.ascontiguousarray(
        mask[b, 0, 0].reshape(16, 128).T.astype(np.float32)
    )
    tri = np.triu(np.ones((128, 128), BF16))

    return {
        "xT": xTd,
        "wqT": wT(Wq),
        "wkT": wT(Wk),
        "wvT": wT(Wv),
        "bqk": bqk,
        "loraA": loraA,
        "qBsT": qBsT,
        "vBa": vBa,
        "amask": amask,
        "tri": tri,
    }


def _run(inputs, trace=False, trace_kwargs=None):
    from concourse.bass_utils import run_bass_kernel_spmd

    args = {k: np.asarray(v) for k, v in inputs.items()}
    in_maps = [
        _prep_core_inputs(
            c,
            args["hidden_states"],
            args["attention_mask"],
            args["Wq"], args["bq"], args["Wk"], args["bk"], args["Wv"], args["bv"],
            args["qA"], args["qB"], args["vA"], args["vB"],
        )
        for c in range(N_CORES)
    ]

    if "nc" not in _cached:
        _cached["nc"] = _build_nc()
    nc = _cached["nc"]

    res = run_bass_kernel_spmd(
        nc, in_maps, core_ids=list(range(N_CORES)), trace=trace,
        **(trace_kwargs or {}),
    )

    full = np.empty((B, T, H), np.float32)
    for c in range(N_CORES):
        b, half = divmod(c, 2)
        full[b, :, half * CH : (half + 1) * CH] = res.results[c]["out"]
    return full, res


def kernel(**inputs):
    full, _ = _run(inputs, trace=False)
    return full
